# revision 10
# baseline (speedup 1.0000x reference)
# Trainium2 Bass kernel for DenseFeatureNumericEmbedding.
#
# Math (per batch row b, feature f):
#   h[b,f,:]  = relu(x[b,f] * W1[f,:] + b1[f,:])          # Linear(1,H) + ReLU
#   emb[b,f,:] = W2[f] @ h[b,f,:] + b2[f,:]               # Linear(H,E)
#   out[b]    = concat_f emb[b,f,:]                       # [B, F*E]
#
# Shapes: B=16384, F=128, H=64, E=16.  8 NeuronCores, batch-sharded (2048 rows/core).
#
# Device pipeline per core (per 1024-row chunk, per feature-pair j = (2j, 2j+1)):
#   1. DMA x (bf16 hi/lo split, column-interleaved) -> PE transpose -> xT [feat, b] in SBUF.
#   2. L1 "broadcast" matmul: 0/1 selector stationary [K=128, M=128] x full xT tile
#      -> PSUM [128p = (2 feats x 64 h-slots), b] = exact fp32 x (hi+lo summed).
#   3. Fused drain at FD=1024 (alternating engines):
#        ACT:  h = relu(scale[p]*x + bias[p])             (per-partition W1/b1 columns)
#        DVE:  h = max(W1[p]*x, -b1[p]) = relu(W1 x + b1) - b1  (residual folded into b2)
#      -> h tiles [128, 1024] bf16 in SBUF.
#   4. L2 matmul: stationary block-diag W2 pair [K=128, M=32] bf16, tile_position
#      col-packed, 4 pairs x 2 halves -> PSUM out [128p = 8 feats x 16 e, 1024b] fp32.
#   5. Drain + b2 bias (per-partition) -> SBUF [fe, b]; PE transpose 128x128 blocks;
#      drain -> out_sb [b, fe] fp32; DMA 1MB contiguous row-blocks to DRAM.
#
# OUT_BF16: route the output transpose through bf16 (cheaper transposes + 2x DVE
# drain of the 16-bit transpose PSUM) at the cost of bf16-rounding the output.

import numpy as np
import ml_dtypes

BF16 = ml_dtypes.bfloat16

B, F, H, E = 16384, 128, 64, 16
NCORES = 8
BC = B // NCORES            # rows per core
CH = 1024                   # batch columns per chunk
FE = F * E                  # output width
NPAIR = F // 2              # feature pairs
NGROUP = F // 8             # groups of 8 features (one out-psum tile each)

OUT_BF16 = False


def _act_pair(j):
    """Pairs whose L1 drain runs on ScalarE (rest on VectorE). Keep in sync with
    the residual fold in _pack_weights."""
    return j % 2 == 0 or j % 16 == 1


def _pack_weights(W1, b1, W2, b2):
    W1 = np.asarray(W1, np.float32)
    b1 = np.asarray(b1, np.float32)
    W2 = np.asarray(W2, np.float32)
    b2 = np.asarray(b2, np.float32)

    # Per-partition L1 scale/bias columns: partition p of pair j holds
    # (feature 2j + p//64, h = p%64).
    scl = np.zeros((128, NPAIR), np.float32)
    bia = np.zeros((128, NPAIR), np.float32)
    for j in range(NPAIR):
        scl[:64, j] = W1[2 * j]
        scl[64:, j] = W1[2 * j + 1]
        bia[:64, j] = b1[2 * j]
        bia[64:, j] = b1[2 * j + 1]

    # L2 stationaries: block-diag per pair, [K=128 (2x64 h), M=32 (2x16 e)].
    w2sb = np.zeros((128, NPAIR * 32), np.float32)
    for j in range(NPAIR):
        w2sb[:64, 32 * j : 32 * j + 16] = W2[2 * j].T          # [H, E]
        w2sb[64:, 32 * j + 16 : 32 * j + 32] = W2[2 * j + 1].T

    # DVE-drained pairs (odd j) produce h' = relu(.) - b1; fold the residual
    # sum_h W2[f,e,h]*b1[f,h] back into the output bias.
    resid = np.einsum("feh,fh->fe", W2, b1)
    b2adj = b2.copy()
    for f in range(F):
        if not _act_pair(f // 2):
            b2adj[f] += resid[f]

    # Output bias columns: partition p of group g = (q=p//32, d=(p%32)//16, e=p%16)
    # -> feature 8g + 2q + d.
    b2col = np.zeros((128, NGROUP), np.float32)
    for g in range(NGROUP):
        for q in range(4):
            for d in range(2):
                f = 8 * g + 2 * q + d
                lo = 32 * q + 16 * d
                b2col[lo : lo + 16, g] = b2adj[f]

    # L1 broadcast selector stationaries, one [K=128, M=128] 0/1 matrix per pair:
    # rows (4j)%128 + (0..3) are the (hi f0, lo f0, hi f1, lo f1) moving rows;
    # out col m<64 -> feat0 (rows 0,1), m>=64 -> feat1 (rows 2,3).
    selq = np.zeros((128, NPAIR * 128), np.float32)
    for j in range(NPAIR):
        p0 = (4 * j) % 128
        m0 = 128 * j
        selq[p0 + 0, m0 : m0 + 64] = 1.0
        selq[p0 + 1, m0 : m0 + 64] = 1.0
        selq[p0 + 2, m0 + 64 : m0 + 128] = 1.0
        selq[p0 + 3, m0 + 64 : m0 + 128] = 1.0

    ident = np.eye(128, dtype=np.float32)
    return dict(
        scl=scl,
        bia=bia,
        bianeg=-bia,
        w2sb=w2sb.astype(BF16),
        b2col=b2col,
        selq=selq.astype(BF16),
        identb=ident.astype(BF16),
        identf=ident,
    )


def _prep_x(xs):
    """Split fp32 x into bf16 hi/lo and column-interleave: col 2f = hi, 2f+1 = lo."""
    xs = np.asarray(xs, np.float32)
    xh = xs.astype(BF16)
    xl = (xs - xh.astype(np.float32)).astype(BF16)
    x_il = np.empty((xs.shape[0], 2 * F), BF16)
    x_il[:, 0::2] = xh
    x_il[:, 1::2] = xl
    return x_il


def _build(nrows):
    from contextlib import ExitStack
    import concourse.bacc as bacc
    import concourse.mybir as mybir
    import concourse.tile as tile

    dt = mybir.dt
    AF = mybir.ActivationFunctionType
    ALU = mybir.AluOpType

    nchunk = nrows // CH
    nsub = CH // 128            # 128-row sub-blocks per chunk
    ot_dt = dt.bfloat16 if OUT_BF16 else dt.float32
    nc = bacc.Bacc(None, target_bir_lowering=False)

    x_il_d = nc.declare_dram_parameter("x_il", [nrows, 2 * F], dt.bfloat16, isOutput=False)
    scl_d = nc.declare_dram_parameter("scl", [128, NPAIR], dt.float32, isOutput=False)
    bia_d = nc.declare_dram_parameter("bia", [128, NPAIR], dt.float32, isOutput=False)
    bianeg_d = nc.declare_dram_parameter("bianeg", [128, NPAIR], dt.float32, isOutput=False)
    w2sb_d = nc.declare_dram_parameter("w2sb", [128, NPAIR * 32], dt.bfloat16, isOutput=False)
    b2col_d = nc.declare_dram_parameter("b2col", [128, NGROUP], dt.float32, isOutput=False)
    selq_d = nc.declare_dram_parameter("selq", [128, NPAIR * 128], dt.bfloat16, isOutput=False)
    identb_d = nc.declare_dram_parameter("identb", [128, 128], dt.bfloat16, isOutput=False)
    identf_d = nc.declare_dram_parameter("identf", [128, 128], dt.float32, isOutput=False)
    out_d = nc.declare_dram_parameter("out", [nrows, FE], dt.float32, isOutput=True)

    with tile.TileContext(nc) as tc, ExitStack() as ctx:
        const = ctx.enter_context(tc.tile_pool(name="const", bufs=1))
        xin_p = ctx.enter_context(tc.tile_pool(name="xin", bufs=2))
        xt_p = ctx.enter_context(tc.tile_pool(name="xt", bufs=2))
        h_p = ctx.enter_context(tc.tile_pool(name="h", bufs=6))
        ot_p = ctx.enter_context(tc.tile_pool(name="ot", bufs=2))
        outsb_p = ctx.enter_context(tc.tile_pool(name="outsb", bufs=1))
        # PSUM budget (8 banks): ps_x 2x[128,1024]f32 = 4, ps_o 2x[128,512]f32 = 2,
        # ps_t2 1x[128,512]f32 = 1, ps_xt 1x[128,1024]bf16 = 1.
        ps_x = ctx.enter_context(tc.tile_pool(name="ps_x", bufs=2, space="PSUM"))
        ps_o = ctx.enter_context(tc.tile_pool(name="ps_o", bufs=2, space="PSUM"))
        ps_t2 = ctx.enter_context(tc.tile_pool(name="ps_t2", bufs=1, space="PSUM"))
        ps_xt = ctx.enter_context(tc.tile_pool(name="ps_xt", bufs=1, space="PSUM"))

        sclT = const.tile([128, NPAIR], dt.float32, tag="scl")
        biaT = const.tile([128, NPAIR], dt.float32, tag="bia")
        bianegT = const.tile([128, NPAIR], dt.float32, tag="bianeg")
        w2T = const.tile([128, NPAIR * 32], dt.bfloat16, tag="w2")
        b2colT = const.tile([128, NGROUP], dt.float32, tag="b2col")
        selqT = const.tile([128, NPAIR * 128], dt.bfloat16, tag="selq")
        identbT = const.tile([128, 128], dt.bfloat16, tag="identb")
        identfT = const.tile([128, 128], dt.float32, tag="identf")
        nc.sync.dma_start(sclT[:], scl_d[:])
        nc.sync.dma_start(biaT[:], bia_d[:])
        nc.sync.dma_start(bianegT[:], bianeg_d[:])
        nc.sync.dma_start(w2T[:], w2sb_d[:])
        nc.sync.dma_start(b2colT[:], b2col_d[:])
        nc.sync.dma_start(selqT[:], selq_d[:])
        nc.sync.dma_start(identbT[:], identb_d[:])
        nc.sync.dma_start(identfT[:], identf_d[:])

        identoT = identbT if OUT_BF16 else identfT

        for c in range(nchunk):
            # --- load + transpose x for this chunk ---
            xin = xin_p.tile([128, nsub, 2 * F], dt.bfloat16, tag="xin")
            src = x_il_d[c * CH : (c + 1) * CH, :].rearrange("(s p) f -> p s f", p=128)
            nc.sync.dma_start(xin[:], src)

            xta = xt_p.tile([128, CH], dt.bfloat16, tag="xta")
            xtb = xt_p.tile([128, CH], dt.bfloat16, tag="xtb")
            for half, xt_dst in ((0, xta), (1, xtb)):
                ps = ps_xt.tile([128, CH], dt.bfloat16, tag="ps_xt")
                for s in range(nsub):
                    nc.tensor.transpose(
                        ps[:, 128 * s : 128 * (s + 1)],
                        xin[:, s, 128 * half : 128 * (half + 1)],
                        identbT[:],
                    )
                nc.vector.tensor_copy(xt_dst[:], ps[:])

            out_sb = outsb_p.tile([128, nsub, NGROUP, 128], dt.float32, tag="out_sb")

            def out_phase(g, hts):
                # L2 matmuls (col-packed 4 pairs per psum half) + bias drain +
                # PE transpose + final drain into out_sb.
                for half in range(2):
                    ps_out = ps_o.tile([128, 512], dt.float32, tag="ps_out")
                    for q in range(4):
                        j = 4 * g + q
                        nc.tensor.matmul(
                            ps_out[32 * q : 32 * q + 32, :],
                            w2T[:, 32 * j : 32 * j + 32],
                            hts[q][:, 512 * half : 512 * (half + 1)],
                            start=True,
                            stop=True,
                            tile_position=(0, 32 * q),
                        )
                    # drain + output bias, [fe, b] orientation
                    ot = ot_p.tile([128, 512], ot_dt, tag="ot")
                    nc.vector.tensor_scalar(
                        ot[:], ps_out[:], b2colT[:, g : g + 1], None, ALU.add
                    )
                    # transpose to [b, fe]: 4 sub-blocks per half
                    pst = ps_t2.tile([128, 4, 128], ot_dt, tag="ps_ot")
                    for t4 in range(4):
                        nc.tensor.transpose(
                            pst[:, t4, :], ot[:, 128 * t4 : 128 * (t4 + 1)], identoT[:]
                        )
                    dst = out_sb[:, 4 * half : 4 * half + 4, g, :]
                    nc.scalar.copy(dst, pst[:])

            pending = None
            for g in range(NGROUP):
                hts = []
                for q in range(4):
                    j = 4 * g + q
                    xt = xta if j < 32 else xtb
                    ps = ps_x.tile([128, CH], dt.float32, tag="ps_x")
                    sel = selqT[:, 128 * j : 128 * (j + 1)]
                    nc.tensor.matmul(
                        ps[:, 0:512], sel, xt[:, 0:512], start=True, stop=True
                    )
                    nc.tensor.matmul(
                        ps[:, 512:1024], sel, xt[:, 512:1024], start=True, stop=True
                    )
                    ht = h_p.tile([128, CH], dt.bfloat16, tag="h")
                    if _act_pair(j):
                        nc.scalar.activation(
                            ht[:],
                            ps[:],
                            AF.Relu,
                            bias=biaT[:, j : j + 1],
                            scale=sclT[:, j : j + 1],
                        )
                    else:
                        nc.vector.tensor_scalar(
                            ht[:],
                            ps[:],
                            sclT[:, j : j + 1],
                            bianegT[:, j : j + 1],
                            ALU.mult,
                            ALU.max,
                        )
                    hts.append(ht)
                if pending is not None:
                    out_phase(*pending)
                pending = (g, hts)
            out_phase(*pending)

            for t in range(nsub):
                r0 = c * CH + t * 128
                nc.sync.dma_start(out_d[r0 : r0 + 128, :], out_sb[:, t, :, :])

    nc.compile()
    return nc


_NC_CACHE = {}


def _get_program(nrows):
    if nrows not in _NC_CACHE:
        _NC_CACHE[nrows] = _build(nrows)
    return _NC_CACHE[nrows]


def kernel(x, W1, b1, W2, b2, _trace=False):
    from concourse.bass_utils import run_bass_kernel_spmd

    cfg = _pack_weights(W1, b1, W2, b2)
    nc = _get_program(BC)
    wkeys = ("scl", "bia", "bianeg", "w2sb", "b2col", "selq", "identb", "identf")
    in_maps = []
    for c in range(NCORES):
        m = {"x_il": _prep_x(x[c * BC : (c + 1) * BC])}
        for k in wkeys:
            m[k] = cfg[k]
        in_maps.append(m)
    res = run_bass_kernel_spmd(
        nc, in_maps, core_ids=list(range(NCORES)), trace=_trace
    )
    out = np.concatenate([r["out"] for r in res.results], axis=0)
    if _trace:
        kernel.last_result = res
    return np.ascontiguousarray(out.astype(np.float32))
